# revision 8
# baseline (speedup 1.0000x reference)
"""Child-Sum TreeLSTM (perfect binary tree, depth 13) on 8 Trainium2 NeuronCores.

Sharding: levels are block-sharded 8 ways. With contiguous block sharding,
children of core p's nodes at level l are exactly core p's nodes at level
l+1, so levels 13(leaves)..3 run with zero communication. One small AllGather
moves the 8 level-3 (h,c) states to every core; levels 2..0 run replicated.

Layout: all state is feature-major [H on partitions (8 blocks of 128), nodes
on the free dim], so child-pair sums and (f*c) pair reductions are stride-2
free-dim vector ops; no transposes anywhere.

Matmuls: float32r (full PE rate at moving free dim >= 256). Gate preacts are
psum = sum_k U_g^T[kb] . h_sum^T[kb]  (+ one K=4 pass  opb_g^T . onehot(op)
which carries the per-node op embedding term and the bias). Leaves use
x = tokens[leaf_token_ids] (host gather), W_g as weights, and fold the
h_init-dependent terms into a per-feature ACT bias.
"""
import os
import numpy as np

H = 1024
D = 1024
NCORES = 8
DEPTH = 13
NLEAF = 2 ** DEPTH
LEAF_PC = NLEAF // NCORES  # 1024
KB = 8

_CACHE = {}


def _host_prep(tokens, leaf_token_ids, op_ids, W_i, W_o, W_u, W_f,
               U_i, U_o, U_u, U_f, b_i, b_o, b_u, b_f,
               op_emb, c_init, h_init):
    f32 = np.float32
    tokens = np.asarray(tokens, f32)
    ids = np.asarray(leaf_token_ids).astype(np.int64)
    ops = np.asarray(op_ids).astype(np.int64)
    W = [np.asarray(w, f32) for w in (W_i, W_o, W_u, W_f)]
    U = [np.asarray(u, f32) for u in (U_i, U_o, U_u, U_f)]
    b = [np.asarray(x, f32).reshape(-1) for x in (b_i, b_o, b_u, b_f)]
    op_emb = np.asarray(op_emb, f32)
    c_init = np.asarray(c_init, f32)
    h_init = np.asarray(h_init, f32)

    leaf_f = bool(np.any(c_init != 0.0))
    ngates = 4 if leaf_f else 3

    x = tokens[ids]
    xT = [np.ascontiguousarray(x[p * LEAF_PC:(p + 1) * LEAF_PC].T)
          for p in range(NCORES)]

    WT = np.ascontiguousarray(
        np.concatenate([W[g].T for g in range(ngates)], axis=1))
    UTiou = np.ascontiguousarray(
        np.concatenate([U[0].T, U[1].T, U[2].T], axis=1))
    UTf = np.ascontiguousarray(U[3].T)

    opb_iou = np.ascontiguousarray(np.concatenate(
        [op_emb @ W[g].T + b[g][None, :] for g in range(3)], axis=1))
    opb_f = np.ascontiguousarray(op_emb @ W[3].T + b[3][None, :])

    hsum0 = h_init.sum(axis=0)
    iou_leaf_bias = np.concatenate([hsum0 @ U[g].T + b[g] for g in range(3)])
    leafb = np.ascontiguousarray(
        iou_leaf_bias.reshape(3, KB, 128).transpose(2, 1, 0))
    f0 = h_init @ U[3].T + b[3][None, :]
    f0rs = np.ascontiguousarray(f0.reshape(2, KB, 128).transpose(2, 1, 0))
    cinitrs = np.ascontiguousarray(c_init.reshape(2, KB, 128).transpose(2, 1, 0))

    lev_ops = {l: ops[2 ** l - 1: 2 ** (l + 1) - 1] for l in range(DEPTH)}
    eye4 = np.eye(4, dtype=f32)

    order = list(range(12, 2, -1)) + [2, 1, 0]
    oh_off = {}
    off = 0
    for l in order:
        m = 2 ** l // NCORES if l >= 3 else 2 ** l
        oh_off[l] = (off, m)
        off += max(m, 2)
    OH_TOT = off

    ohA, ohxA = [], []
    for p in range(NCORES):
        cols = []
        for l in order:
            o = lev_ops[l]
            if l >= 3:
                m = 2 ** l // NCORES
                o = o[p * m:(p + 1) * m]
            if len(o) == 1:
                o = np.concatenate([o, o])
            cols.append(eye4[o].T)
        ohp = np.concatenate(cols, axis=1)
        ohA.append(np.ascontiguousarray(ohp))
        ohxA.append(np.ascontiguousarray(np.repeat(ohp, 2, axis=1)))

    return dict(xT=xT, WT=WT, UTiou=UTiou, UTf=UTf, opb_iou=opb_iou,
                opb_f=opb_f, leafb=leafb, f0rs=f0rs, cinitrs=cinitrs,
                ohA=ohA, ohxA=ohxA, oh_off=oh_off, OH_TOT=OH_TOT,
                leaf_f=leaf_f, ngates=ngates)


def _build_bass(leaf_f, ngates, OH_TOT, oh_off, debug_taps=False):
    from contextlib import ExitStack

    import concourse.mybir as mybir
    import concourse.tile as tile
    from concourse import bacc

    f32 = mybir.dt.float32
    f32r = mybir.dt.float32r
    AF = mybir.ActivationFunctionType

    nc = bacc.Bacc("TRN2", target_bir_lowering=False, debug=False,
                   num_devices=NCORES)

    xT_d = nc.dram_tensor("xT", [D, LEAF_PC], f32r, kind="ExternalInput").ap()
    WT_d = nc.dram_tensor("WT", [D, ngates * H], f32r, kind="ExternalInput").ap()
    UTiou_d = nc.dram_tensor("UTiou", [H, 3 * H], f32r, kind="ExternalInput").ap()
    UTf_d = nc.dram_tensor("UTf", [H, H], f32r, kind="ExternalInput").ap()
    opb_iou_d = nc.dram_tensor("opb_iou", [4, 3 * H], f32r,
                               kind="ExternalInput").ap()
    opb_f_d = nc.dram_tensor("opb_f", [4, H], f32r, kind="ExternalInput").ap()
    leafb_d = nc.dram_tensor("leafb", [128, KB, 3], f32, kind="ExternalInput").ap()
    ohA_d = nc.dram_tensor("ohA", [4, OH_TOT], f32r, kind="ExternalInput").ap()
    ohxA_d = nc.dram_tensor("ohxA", [4, 2 * OH_TOT], f32r,
                            kind="ExternalInput").ap()
    if leaf_f:
        f0rs_d = nc.dram_tensor("f0rs", [128, KB, 2], f32,
                                kind="ExternalInput").ap()
        cinitrs_d = nc.dram_tensor("cinitrs", [128, KB, 2], f32,
                                   kind="ExternalInput").ap()
    out_d = nc.dram_tensor("out_root", [2, H], f32, kind="ExternalOutput").ap()

    tap_kind = "ExternalOutput" if debug_taps else "Internal"
    h13d = nc.dram_tensor("h13d", [128, KB, LEAF_PC], f32r, kind=tap_kind).ap()
    c13d = nc.dram_tensor("c13d", [128, KB, LEAF_PC], f32, kind=tap_kind).ap()
    h12d = nc.dram_tensor("h12d", [128, KB, 512], f32r, kind=tap_kind).ap()
    c12d = nc.dram_tensor("c12d", [128, KB, 512], f32, kind=tap_kind).ap()
    h11d = nc.dram_tensor("h11d", [128, KB, 256], f32r, kind=tap_kind).ap()
    c11d = nc.dram_tensor("c11d", [128, KB, 256], f32, kind=tap_kind).ap()
    tapd = {}
    if debug_taps:
        for l in list(range(10, 2, -1)) + [2, 1, 0]:
            m = 2 ** l // NCORES if l >= 3 else 2 ** l
            tapd[l] = (
                nc.dram_tensor(f"h{l}t", [128, KB, m], f32r,
                               kind="ExternalOutput").ap(),
                nc.dram_tensor(f"c{l}t", [128, KB, m], f32,
                               kind="ExternalOutput").ap(),
            )

    with tile.TileContext(nc) as tc, ExitStack() as top:
        const = top.enter_context(tc.tile_pool(name="const", bufs=1))
        psA = top.enter_context(tc.tile_pool(name="psA", bufs=8, space="PSUM"))
        dram = top.enter_context(tc.tile_pool(name="dram", bufs=1, space="DRAM"))

        UTiou_sb = const.tile([128, KB, 3 * H], f32r)
        for kb in range(KB):
            nc.sync.dma_start(out=UTiou_sb[:, kb, :],
                              in_=UTiou_d[kb * 128:(kb + 1) * 128, :])
        leafb_sb = const.tile([128, KB, 3], f32)
        nc.sync.dma_start(out=leafb_sb, in_=leafb_d)
        if leaf_f:
            f0_sb = const.tile([128, KB, 2], f32)
            nc.sync.dma_start(out=f0_sb, in_=f0rs_d)
            ci_sb = const.tile([128, KB, 2], f32)
            nc.sync.dma_start(out=ci_sb, in_=cinitrs_d)

        # ---------------- leaves (level 13) ----------------
        with ExitStack() as lf:
            lp_x = lf.enter_context(tc.tile_pool(name="lp_x", bufs=1))
            lp_w = lf.enter_context(tc.tile_pool(name="lp_w", bufs=3))
            lp_s = lf.enter_context(tc.tile_pool(name="lp_s", bufs=2))

            xT_sb = lp_x.tile([128, KB, LEAF_PC], f32r)
            for kb in range(KB):
                nc.sync.dma_start(out=xT_sb[:, kb, :],
                                  in_=xT_d[kb * 128:(kb + 1) * 128, :])

            for fb in range(KB):
                wts = []
                for g in range(ngates):
                    wt = lp_w.tile([128, KB, 128], f32r, name=f"wt{fb}{g}",
                                   tag="wt")
                    col = g * H + fb * 128
                    nc.sync.dma_start(
                        out=wt, in_=WT_d[:, col:col + 128].rearrange(
                            "(kb p) m -> p kb m", p=128))
                    wts.append(wt)
                for ch in range(2):  # leaf node chunks of 512
                    n0 = ch * 512
                    gates = []
                    for g in range(ngates):
                        ps = psA.tile([128, 512], f32, name=f"lps{fb}{g}{ch}",
                                      tag="ps")
                        for kb in range(KB):
                            nc.tensor.matmul(ps, wts[g][:, kb, :],
                                             xT_sb[:, kb, n0:n0 + 512],
                                             start=(kb == 0),
                                             stop=(kb == KB - 1))
                        if g < 3:
                            gt = lp_s.tile([128, 512], f32,
                                           name=f"lg{fb}{g}{ch}", tag=f"lg{g}")
                            nc.scalar.activation(
                                gt, ps, AF.Tanh if g == 2 else AF.Sigmoid,
                                bias=leafb_sb[:, fb, g:g + 1])
                        else:
                            gt = ps  # keep f preact in psum
                        gates.append(gt)
                    cn = lp_s.tile([128, 512], f32, name=f"lc{fb}{ch}", tag="lc")
                    nc.vector.tensor_mul(cn, gates[0], gates[2])
                    if leaf_f:
                        for child in range(2):
                            fg = lp_s.tile([128, 512], f32,
                                           name=f"lf{fb}{ch}{child}", tag="lf")
                            nc.scalar.activation(
                                fg, gates[3], AF.Sigmoid,
                                bias=f0_sb[:, fb, child:child + 1])
                            nc.vector.tensor_scalar(
                                fg, fg, ci_sb[:, fb, child:child + 1], None,
                                mybir.AluOpType.mult)
                            nc.vector.tensor_add(cn, cn, fg)
                    tcf = lp_s.tile([128, 512], f32, name=f"lt{fb}{ch}", tag="lt")
                    nc.scalar.activation(tcf, cn, AF.Tanh)
                    hn = lp_s.tile([128, 512], f32r, name=f"lh{fb}{ch}", tag="lh")
                    nc.vector.tensor_mul(hn, gates[1], tcf)
                    nc.sync.dma_start(out=h13d[:, fb, n0:n0 + 512], in_=hn)
                    nc.sync.dma_start(out=c13d[:, fb, n0:n0 + 512], in_=cn)

        # ---------------- internal levels ----------------
        const2 = top.enter_context(tc.tile_pool(name="const2", bufs=1))
        opb_iou_sb = const2.tile([4, 3 * H], f32r)
        nc.sync.dma_start(out=opb_iou_sb, in_=opb_iou_d)
        opb_f_sb = const2.tile([4, H], f32r)
        nc.sync.dma_start(out=opb_f_sb, in_=opb_f_d)

        states = top.enter_context(tc.tile_pool(name="states", bufs=1))
        lvl = top.enter_context(tc.tile_pool(name="lvl", bufs=2))
        ohp = top.enter_context(tc.tile_pool(name="ohp", bufs=1))
        big = top.enter_context(tc.tile_pool(name="big", bufs=1))

        def emit_level(l, m, h_src, c_src, h_dst, c_dst, src_dram):
            """One Child-Sum level, feature-major. h_src/c_src: APs (DRAM or
            SBUF) shaped [128, KB, 2m]; dst likewise [128, KB, m] (or None ->
            allocate SBUF state tiles and return them)."""
            off, m_chk = oh_off[l]
            assert m == m_chk
            dst_dram = h_dst is not None

            ma = max(m, 2)
            ohl = ohp.tile([4, ma], f32r, name=f"oh{l}", tag="ohl")
            nc.sync.dma_start(out=ohl, in_=ohA_d[:, off:off + ma])
            ohxl = ohp.tile([4, 2 * m], f32r, name=f"ohx{l}", tag="ohxl")
            nc.sync.dma_start(out=ohxl, in_=ohxA_d[:, 2 * off:2 * off + 2 * m])


            if not dst_dram:
                h_out = states.tile([128, KB, m], f32r, name=f"h{l}s",
                                    tag=f"h{l}s")
                c_out = states.tile([128, KB, m], f32, name=f"c{l}s",
                                    tag=f"c{l}s")
            else:
                h_out = c_out = None

            cc = min(512, 2 * m)       # child columns per chunk
            nchunks = (2 * m) // cc
            nn = cc // 2               # output nodes per chunk
            nnp = max(nn, 2)           # fp32r needs even moving free dims

            for ci in range(nchunks):
                c0 = ci * cc
                n0 = ci * nn
                if src_dram:
                    hch = big.tile([128, KB, cc], f32r, name=f"hch{l}{ci}",
                                   tag="hch", bufs=1)
                    nc.sync.dma_start(out=hch, in_=h_src[:, :, c0:c0 + cc])
                else:
                    hch = h_src[:, :, c0:c0 + cc]
                hs = big.tile([128, KB, nnp], f32r, name=f"hs{l}{ci}", tag="hs",
                              bufs=1)
                hv = hch.rearrange("p k (n two) -> p k n two", two=2)
                nc.vector.tensor_add(hs[:, :, :nn], hv[:, :, :, 0],
                                     hv[:, :, :, 1])
                if nnp != nn:
                    nc.vector.tensor_copy(hs[:, :, nn:nnp], hs[:, :, :nnp - nn])

                for fb in range(KB):
                    if src_dram:
                        cchf = lvl.tile([128, cc], f32, name=f"cch{l}{ci}{fb}",
                                        tag="cch")
                        nc.sync.dma_start(out=cchf, in_=c_src[:, fb, c0:c0 + cc])
                    else:
                        cchf = c_src[:, fb, c0:c0 + cc]
                    utf = lvl.tile([128, KB, 128], f32r, name=f"utf{l}{ci}{fb}",
                                   tag="utf", bufs=1)
                    nc.sync.dma_start(
                        out=utf, in_=UTf_d[:, fb * 128:fb * 128 + 128].rearrange(
                            "(kb p) m -> p kb m", p=128))

                    gates = []
                    for g in range(3):
                        ps = psA.tile([128, nnp], f32, name=f"ps{l}{ci}{fb}{g}",
                                      tag="ps", padded_shape=[128, 512])
                        col = g * H + fb * 128
                        for kb in range(KB):
                            nc.tensor.matmul(ps, UTiou_sb[:, kb, col:col + 128],
                                             hs[:, kb, :], start=(kb == 0),
                                             stop=False)
                        nc.tensor.matmul(ps, opb_iou_sb[:, col:col + 128],
                                         ohl[:, n0:n0 + nnp], start=False,
                                         stop=True)
                        gt = lvl.tile([128, nn], f32, name=f"g{l}{ci}{fb}{g}",
                                      tag=f"g{g}")
                        nc.scalar.activation(gt, ps[:, :nn],
                                             AF.Tanh if g == 2 else AF.Sigmoid)
                        gates.append(gt)

                    psf = psA.tile([128, cc], f32, name=f"psf{l}{ci}{fb}",
                                   tag="ps", padded_shape=[128, 512])
                    for kb in range(KB):
                        nc.tensor.matmul(psf, utf[:, kb, :], hch[:, kb, :],
                                         start=(kb == 0), stop=False)
                    nc.tensor.matmul(psf, opb_f_sb[:, fb * 128:fb * 128 + 128],
                                     ohxl[:, 2 * n0:2 * n0 + cc], start=False,
                                     stop=True)
                    ft = lvl.tile([128, cc], f32, name=f"ft{l}{ci}{fb}", tag="ft")
                    nc.scalar.activation(ft, psf, AF.Sigmoid)

                    fc = ft
                    nc.vector.tensor_mul(fc, ft, cchf)

                    if dst_dram:
                        cn = lvl.tile([128, nn], f32, name=f"cn{l}{ci}{fb}",
                                      tag="cn")
                    else:
                        cn = c_out[:, fb, n0:n0 + nn]
                    fv = fc.rearrange("p (n two) -> p n two", two=2)
                    nc.vector.tensor_mul(cn, gates[0], gates[2])
                    nc.vector.tensor_add(cn, cn, fv[:, :, 0])
                    nc.vector.tensor_add(cn, cn, fv[:, :, 1])

                    tcf = lvl.tile([128, nn], f32, name=f"tc{l}{ci}{fb}",
                                   tag="tcf")
                    nc.scalar.activation(tcf, cn, AF.Tanh)
                    if dst_dram:
                        hn = lvl.tile([128, nn], f32r, name=f"hn{l}{ci}{fb}",
                                      tag="hn")
                        nc.vector.tensor_mul(hn, gates[1], tcf)
                        nc.sync.dma_start(out=h_dst[:, fb, n0:n0 + nn], in_=hn)
                        nc.sync.dma_start(out=c_dst[:, fb, n0:n0 + nn], in_=cn)
                    else:
                        nc.vector.tensor_mul(h_out[:, fb, n0:n0 + nn],
                                             gates[1], tcf)
            if not dst_dram and debug_taps and l in tapd:
                nc.sync.dma_start(out=tapd[l][0], in_=h_out)
                nc.sync.dma_start(out=tapd[l][1], in_=c_out)
            return h_out, c_out

        # level 12: DRAM -> DRAM
        emit_level(12, 512, h13d, c13d, h12d, c12d, src_dram=True)
        if debug_taps:
            pass  # h12d/c12d already external
        # level 11: DRAM -> SBUF
        emit_level(11, 256, h12d, c12d, h11d, c11d, src_dram=True)
        # level 10: DRAM -> SBUF; levels 9..3: SBUF -> SBUF
        h_cur, c_cur = emit_level(10, 128, h11d, c11d, None, None, src_dram=True)
        for l in range(9, 2, -1):
            m = 2 ** l // NCORES
            h_cur, c_cur = emit_level(l, m, h_cur, c_cur, None, None, src_dram=False)

        # ---- AllGather of the eight level-3 (h,c) states ----
        agin = dram.tile([1, 2 * H], f32r)
        agout = dram.tile([NCORES, 2 * H], f32r)
        nc.sync.dma_start(
            out=agin[0, :H].rearrange("(kb p) -> p kb", p=128),
            in_=h_cur[:, :, 0])
        nc.sync.dma_start(
            out=agin[0, H:].rearrange("(kb p) -> p kb", p=128),
            in_=c_cur[:, :, 0].bitcast(f32r))
        nc.gpsimd.collective_compute(
            "AllGather", mybir.AluOpType.bypass,
            replica_groups=[list(range(NCORES))],
            ins=[agin.opt()], outs=[agout.opt()])
        h3f = states.tile([128, KB, NCORES], f32r)
        c3f = states.tile([128, KB, NCORES], f32)
        for n in range(NCORES):
            nc.sync.dma_start(
                out=h3f[:, :, n],
                in_=agout[n, :H].rearrange("(kb p) -> p kb", p=128))
            nc.sync.dma_start(
                out=c3f[:, :, n],
                in_=agout[n, H:].rearrange("(kb p) -> p kb", p=128).bitcast(f32))

        # replicated top levels 2..0
        h_cur, c_cur = h3f, c3f
        for l in (2, 1, 0):
            h_cur, c_cur = emit_level(l, 2 ** l, h_cur, c_cur, None, None, src_dram=False)

        nc.sync.dma_start(
            out=out_d[0, :].rearrange("(kb p) -> p kb", p=128),
            in_=c_cur[:, :, 0])
        nc.sync.dma_start(
            out=out_d[1, :].rearrange("(kb p) -> p kb", p=128),
            in_=h_cur[:, :, 0].bitcast(f32))

    nc.compile()
    return nc


def kernel(**inputs):
    hp = _host_prep(**inputs)
    debug_taps = bool(int(os.environ.get("TREE_DEBUG_TAPS", "0")))
    key = (hp["leaf_f"], hp["ngates"], debug_taps)
    if key not in _CACHE:
        _CACHE[key] = _build_bass(hp["leaf_f"], hp["ngates"], hp["OH_TOT"],
                                  hp["oh_off"], debug_taps)
    nc = _CACHE[key]

    shared = {"WT": hp["WT"], "UTiou": hp["UTiou"], "UTf": hp["UTf"],
              "opb_iou": hp["opb_iou"], "opb_f": hp["opb_f"],
              "leafb": hp["leafb"]}
    if hp["leaf_f"]:
        shared["f0rs"] = hp["f0rs"]
        shared["cinitrs"] = hp["cinitrs"]
    in_maps = []
    for p in range(NCORES):
        m = dict(shared)
        m["xT"] = hp["xT"][p]
        m["ohA"] = hp["ohA"][p]
        m["ohxA"] = hp["ohxA"][p]
        in_maps.append(m)

    from concourse.bass_utils import run_bass_kernel_spmd
    trace = bool(int(os.environ.get("TREE_TRACE", "0")))
    if trace:
        import axon_trace_shim  # noqa: F401
    r = run_bass_kernel_spmd(nc, in_maps, core_ids=list(range(NCORES)),
                             trace=trace)
    kernel.last_result = r
    out = r.results[0]["out_root"]  # [2, H]
    return np.ascontiguousarray(out[:, None, :]).astype(np.float32)
